# revision 3
# baseline (speedup 1.0000x reference)
"""AttentionPooling (segment softmax-pool) Trainium2 Bass kernel.

out[g, :] = sum_{i: batch[i]==g} softmax_within_segment(score)_i * x[i, :]
score_i = tanh(x_i @ W1 + b1) @ W2 + b2

Math notes:
- softmax is shift-invariant: b2 and the per-segment max subtraction cancel
  exactly, so e_i = exp(s_i) with s_i = tanh(xW1+b1)@W2 and the per-segment
  sum of e normalizes at the very end.
- per-segment sums run on the TensorEngine: for each 128-node subtile a
  one-hot matrix ow[i, g] = e_i * (batch_local[i] == g) is built with one
  DVE tensor_scalar op, then ow.T @ [x | 1] accumulates [GM, D+1] into a
  single PSUM tile that lives across the WHOLE kernel (column D is the
  softmax denominator); one normalization pass at the end.

v2 layout: x is loaded twice (natural for pooling, transposed for the score
MLP) but in fp16 with host-side layouts giving 4KB-contiguous per-partition
DMA descriptors and one dma_start per chunk per copy. All per-chunk PSUM
eviction work from v1 is gone.

Sharding: nodes split across 8 cores at segment boundaries (batch is
sorted); each core reduces its own segments; host concatenates the
per-core [G_c, D] outputs.
"""

import sys

sys.path.insert(0, "/opt/trn_rl_repo")

import numpy as np

import concourse.bass as bass
import concourse.tile as tile
from concourse import mybir
from concourse.bass_utils import run_bass_kernel_spmd

F16 = np.float16

N_CORES = 8
D = 256
H = 128  # hidden dim of the score MLP
C = 1024  # nodes per chunk
S = C // 128  # subtiles per chunk


def _split_multiwait(nc):
    """Split multi-wait instructions for this walrus build.

    This neuronxcc/walrus rejects more than one sync-wait command per
    instruction, but tile emits 2-3 waits on compute/DMA instructions and
    many on the final Drain. Hoist the extra waits onto preceding
    InstEventSemaphore instructions (the native sequencer wait primitive,
    2 waits each) on the same engine. Engine program order makes this
    equivalent: the stream blocks on the EVSEM waits, then on the
    instruction's remaining wait.
    """
    for bb in nc.main_func.blocks:
        new = []
        for ins in bb.instructions:
            w = (
                list(ins.sync_info.on_wait)
                if (ins.sync_info and ins.sync_info.on_wait)
                else []
            )
            if len(w) > 1:
                extras = w[:-1]
                for i in range(0, len(extras), 2):
                    ev = mybir.InstEventSemaphore(
                        name=nc.get_next_instruction_name(),
                        engine=ins.engine,
                        sync_info=mybir.SyncInfo(
                            on_wait=extras[i : i + 2], on_update=[]
                        ),
                    )
                    nc.register_instruction(ev)
                    new.append(ev)
                ins.sync_info.on_wait = [w[-1]]
            new.append(ins)
        bb.instructions[:] = new


def _build_program(T, GM):
    """SPMD Bass program: T chunks of C nodes, GM local segments."""
    f32 = mybir.dt.float32
    f16 = mybir.dt.float16

    nc = bass.Bass(trn_type="TRN2")
    # host layouts (see _prepare): node-partition-major natural copy with a
    # trailing ones column baked in, and a d-major transposed copy.
    xnh = nc.dram_tensor("xnh", [128, T, S, D + 1], f16, kind="ExternalInput")
    xth = nc.dram_tensor("xth", [128, T, 2, C], f16, kind="ExternalInput")
    w1 = nc.dram_tensor("w1", [2, 128, H], f16, kind="ExternalInput")
    w2 = nc.dram_tensor("w2", [H, 1], f16, kind="ExternalInput")
    b1v = nc.dram_tensor("b1v", [H, 1], f32, kind="ExternalInput")
    blo = nc.dram_tensor("blo", [128, T * S], f32, kind="ExternalInput")
    iot = nc.dram_tensor("iot", [128, GM], f16, kind="ExternalInput")
    dbi = nc.dram_tensor("dbi", [GM, 1], f32, kind="ExternalInput")
    out = nc.dram_tensor("out", [GM, D], f32, kind="ExternalOutput")

    Exp = mybir.ActivationFunctionType.Exp
    Tanh = mybir.ActivationFunctionType.Tanh

    with tile.TileContext(nc) as tc:
        with (
            tc.tile_pool(name="const", bufs=1) as const,
            tc.tile_pool(name="work", bufs=3) as work,
            tc.tile_pool(name="poolacc", bufs=1, space="PSUM") as poolacc,
            tc.tile_pool(name="hps", bufs=2, space="PSUM") as hps,
            tc.tile_pool(name="sps", bufs=2, space="PSUM") as sps,
        ):
            w1sb = const.tile([128, 2, H], f16)
            nc.sync.dma_start(out=w1sb[:, 0, :], in_=w1[0])
            nc.sync.dma_start(out=w1sb[:, 1, :], in_=w1[1])
            w2sb = const.tile([H, 1], f16)
            nc.sync.dma_start(out=w2sb, in_=w2[:, :])
            b1sb = const.tile([H, 1], f32)
            nc.sync.dma_start(out=b1sb, in_=b1v[:, :])
            blsb = const.tile([128, T * S], f32)
            nc.sync.dma_start(out=blsb, in_=blo[:, :])
            iosb = const.tile([128, GM], f16)
            nc.sync.dma_start(out=iosb, in_=iot[:, :])
            dbsb = const.tile([GM, 1], f32)
            nc.sync.dma_start(out=dbsb, in_=dbi[:, :])

            # single PSUM accumulator for the pooled sums, alive across the
            # whole kernel: memset once, every pool matmul accumulates with
            # start=False (the baseline-proven memset+accumulate pattern).
            pooled = poolacc.tile([GM, D + 1], f32)
            nc.vector.memset(pooled, 0.0)

            for j in range(T):
                # one DMA per copy per chunk; 4KB contiguous per partition
                xtt = work.tile([128, 2, C], f16)
                nc.sync.dma_start(out=xtt, in_=xth[:, j, :, :])
                xnt = work.tile([128, S, D + 1], f16)
                nc.sync.dma_start(out=xnt, in_=xnh[:, j, :, :])

                # hT[hidden, node] = sum_d W1[d, hidden] * x[node, d]
                # (moving operand capped at 512 columns per ISA check)
                hp = hps.tile([H, C], f32)
                for q in range(0, C, 512):
                    nc.tensor.matmul(
                        hp[:, q : q + 512],
                        lhsT=w1sb[:, 0, :],
                        rhs=xtt[:, 0, q : q + 512],
                        start=True,
                        stop=False,
                        skip_group_check=True,
                    )
                    nc.tensor.matmul(
                        hp[:, q : q + 512],
                        lhsT=w1sb[:, 1, :],
                        rhs=xtt[:, 1, q : q + 512],
                        start=False,
                        stop=True,
                        skip_group_check=True,
                    )
                tht = work.tile([H, C], f16)
                nc.scalar.activation(tht, hp, Tanh, bias=b1sb[:, 0:1])

                # scores: the S matmuls share sp's PSUM region, so zero it
                # explicitly and accumulate (start=False) to avoid the
                # whole-region clear/tag of start=True racing neighbors.
                sp = sps.tile([128, S], f32)
                nc.vector.memset(sp, 0.0)
                et = work.tile([128, S], f32)
                owt = work.tile([128, S, GM], f16)
                for a in range(S):
                    # s[node] = sum_h tanh_h[h, node] * W2[h]
                    nc.tensor.matmul(
                        sp[:, a : a + 1],
                        lhsT=tht[:, a * 128 : (a + 1) * 128],
                        rhs=w2sb,
                        start=False,
                        stop=False,
                        skip_group_check=True,
                    )
                nc.scalar.activation(et, sp, Exp)
                for a in range(S):
                    # ow[i, g] = (iota[g] == batch_local[i]) * e[i]
                    t4 = j * S + a
                    nc.vector.tensor_scalar(
                        out=owt[:, a, :],
                        in0=iosb,
                        scalar1=blsb[:, t4 : t4 + 1],
                        scalar2=et[:, a : a + 1],
                        op0=mybir.AluOpType.is_equal,
                        op1=mybir.AluOpType.mult,
                    )
                    nc.tensor.matmul(
                        pooled,
                        lhsT=owt[:, a, :],
                        rhs=xnt[:, a, :],
                        start=False,
                        stop=False,
                        skip_group_check=True,
                    )

            # normalize: out[g, :] = pooled[g, :D] / (denom[g] + empty_guard)
            dn = work.tile([GM, 1], f32)
            nc.vector.tensor_scalar_add(dn, pooled[:, D : D + 1], dbsb[:, 0:1])
            rc = work.tile([GM, 1], f32)
            nc.vector.reciprocal(rc, dn)
            ot = work.tile([GM, D], f32)
            nc.vector.tensor_scalar_mul(ot, pooled[:, 0:D], rc[:, 0:1])
            nc.sync.dma_start(out=out[:, :], in_=ot)

    _split_multiwait(nc)
    return nc


def _prepare(inputs):
    """Host-side sharding and input staging. Returns (meta, in_maps)."""
    x = np.asarray(inputs["x"], dtype=np.float32)
    batch = np.asarray(inputs["batch"]).astype(np.int64)
    W1 = np.asarray(inputs["W1"], dtype=np.float32)
    b1 = np.asarray(inputs["b1"], dtype=np.float32)
    W2 = np.asarray(inputs["W2"], dtype=np.float32)

    n, d = x.shape
    assert d == D
    G = 512
    seg_ptr = np.searchsorted(batch, np.arange(G + 1))  # [G+1], seg g rows

    # split at segment boundaries, balancing rows
    targets = (np.arange(N_CORES + 1) * n) // N_CORES
    g_bounds = np.zeros(N_CORES + 1, dtype=np.int64)
    g_bounds[N_CORES] = G
    for c in range(1, N_CORES):
        g = int(np.argmin(np.abs(seg_ptr.astype(np.int64) - targets[c])))
        g_bounds[c] = max(g, g_bounds[c - 1])
    row_bounds = seg_ptr[g_bounds]

    rows = np.diff(row_bounds)
    segs = np.diff(g_bounds)
    GM = int(segs.max())
    assert GM <= 128, f"too many segments on one core: {GM}"
    T = int(-(-int(rows.max()) // C))
    R = T * C

    xh = x.astype(F16)
    w1s = np.ascontiguousarray(W1.reshape(2, 128, H).astype(F16))
    w2s = np.ascontiguousarray(W2.astype(F16))
    b1s = np.ascontiguousarray(b1.reshape(H, 1))
    iota = np.broadcast_to(
        np.arange(GM, dtype=F16)[None, :], (128, GM)
    ).copy()

    in_maps = []
    for c in range(N_CORES):
        r0, r1 = int(row_bounds[c]), int(row_bounds[c + 1])
        g0, g1 = int(g_bounds[c]), int(g_bounds[c + 1])
        nr = r1 - r0
        xpad = np.zeros((R, D), dtype=F16)
        xpad[:nr] = xh[r0:r1]
        # natural copy, node-partition-major, ones column baked in:
        # xnh[p, j, a, :D] = x[node j*C + a*128 + p], xnh[..., D] = 1
        xnh = np.ones((128, T, S, D + 1), dtype=F16)
        xnh[:, :, :, :D] = xpad.reshape(T, S, 128, D).transpose(2, 0, 1, 3)
        # transposed copy: xth[p, j, cc, i] = x[node j*C + i, cc*128 + p]
        xth = np.ascontiguousarray(
            xpad.reshape(T, C, 2, 128).transpose(3, 0, 2, 1)
        )
        blo = np.full((128, T * S), -1.0, dtype=np.float32)
        blp = np.full(R, -1.0, dtype=np.float32)
        blp[:nr] = (batch[r0:r1] - g0).astype(np.float32)
        blo[:, :] = blp.reshape(T * S, 128).T
        # 1.0 for empty or padded segments (their denominator is 0)
        seg_count = np.zeros(GM, dtype=np.int64)
        cnts = seg_ptr[g0 + 1 : g1 + 1] - seg_ptr[g0:g1]
        seg_count[: g1 - g0] = cnts
        dbi = (seg_count == 0).astype(np.float32).reshape(GM, 1)
        in_maps.append(
            {
                "xnh": xnh,
                "xth": xth,
                "w1": w1s,
                "w2": w2s,
                "b1v": b1s,
                "blo": blo,
                "iot": iota,
                "dbi": dbi,
            }
        )

    meta = {
        "T": T,
        "GM": GM,
        "g_bounds": g_bounds,
        "G": G,
        "n": n,
    }
    return meta, in_maps


def _run(inputs, trace=False):
    meta, in_maps = _prepare(inputs)
    nc = _build_program(meta["T"], meta["GM"])
    try:
        res = run_bass_kernel_spmd(nc, in_maps, list(range(N_CORES)), trace=trace)
    except Exception:
        # transient device failures (e.g. NRT_EXEC_UNIT_UNRECOVERABLE) happen;
        # one rebuild+retry
        nc = _build_program(meta["T"], meta["GM"])
        res = run_bass_kernel_spmd(nc, in_maps, list(range(N_CORES)), trace=trace)
    G = meta["G"]
    g_bounds = meta["g_bounds"]
    full = np.zeros((G, D), dtype=np.float32)
    for c in range(N_CORES):
        g0, g1 = int(g_bounds[c]), int(g_bounds[c + 1])
        if g1 > g0:
            full[g0:g1] = res.results[c]["out"][: g1 - g0]
    return full, res


def kernel(**inputs) -> np.ndarray:
    out, _ = _run(inputs, trace=False)
    return out


def kernel_traced(**inputs):
    """Returns (output, BassKernelResults with exec_time_ns/profile)."""
    out, res = _run(inputs, trace=True)
    return out, res
